# revision 11
# baseline (speedup 1.0000x reference)
"""MoE routing kernel for Trainium2 (8 NeuronCores, expert-parallel).

out[i] = x[i] + relu(x[i] @ W[e].T + b[e]),  e = cam_pred_ids[i]

Strategy: route tokens by expert on the host (the sharding step), so core e
computes ONLY expert e's tokens with ONLY W[e] (16MB instead of 128MB).
All device-side layouts are pre-transposed on the host so every DMA is
contiguous: the device computes hT[o, n] = sum_k WT[k, o] * xT[k, n] with
K on partitions for both operands, then outT = xT + relu(hT + b) and the
host transposes/scatters back.
"""

import os
import numpy as np

import concourse.bass as bass
from concourse import bacc
import concourse.mybir as mybir
import concourse.tile as tile
from concourse.bass_utils import run_bass_kernel_spmd

NUM_EXPERTS = 8
DIM = 2048
KT = DIM // 128  # 16 k-tiles
OT = DIM // 128  # 16 o-tiles

# matmul operand dtype mode: 'f32r' (fp32 data, 1 cyc/row), 'f32' (exact, 4
# cyc/row), 'f16' / 'bf16' (16-bit operands, 1 cyc/row)
MODE = os.environ.get("BASS_MOE_MODE", "f32r")


def _chunks(np_tokens: int) -> list[tuple[int, int]]:
    """Split the free dim into matmul chunks of <=512, each >=256 when
    possible (float32r runs 4x slower below 256 moving columns)."""
    out = []
    pos = 0
    rem = np_tokens
    while rem > 0:
        if rem > 512 + 256:
            take = 512
        elif rem > 512:
            take = (rem + 1) // 2  # two chunks, both >=256
        else:
            take = rem
        out.append((pos, take))
        pos += take
        rem -= take
    return out


def _build_nc(np_tokens: int, mode: str):
    f32 = mybir.dt.float32
    mm_dt = {
        "f32r": mybir.dt.float32r,
        "f32": mybir.dt.float32,
        "f16": mybir.dt.float16,
        "bf16": mybir.dt.bfloat16,
    }[mode]
    sixteen_bit = mode in ("f16", "bf16")

    # For f32/f32r the x input doubles as matmul rhs and residual; walrus
    # requires fp32r matmul operands to be produced as fp32r, so the tiles
    # carry mm_dt and get bitcast to f32 for the residual add (same bits).
    xt_dt = mm_dt if mode in ("f32", "f32r") else f32

    nc = bacc.Bacc()
    wt_d = nc.declare_dram_parameter("wt", [OT, 128, KT, 128], mm_dt, isOutput=False)
    xt_d = nc.declare_dram_parameter("xt", [DIM, np_tokens], xt_dt, isOutput=False)
    if sixteen_bit:
        xtm_d = nc.declare_dram_parameter(
            "xtm", [DIM, np_tokens], mm_dt, isOutput=False
        )
    b_d = nc.declare_dram_parameter("b", [128, OT], f32, isOutput=False)
    out_d = nc.declare_dram_parameter("out", [DIM, np_tokens], f32, isOutput=True)

    chunks = _chunks(np_tokens)
    relu = mybir.ActivationFunctionType.Relu

    with tile.TileContext(nc) as tc:
        with (
            tc.tile_pool(name="xp", bufs=1) as xp,
            tc.tile_pool(name="wp", bufs=3) as wp,
            tc.tile_pool(name="op", bufs=3) as op,
            tc.tile_pool(name="bp", bufs=1) as bp,
            tc.tile_pool(name="pp", bufs=2, space="PSUM") as pp,
        ):
            # First weight tile is DMA'd BEFORE the x loads (split in pieces
            # so the first matmul group can start on kt=0 early): queue
            # semaphore targets accumulate in program order, so anything
            # queued ahead delays the first matmul's wait from clearing.
            wtiles = {}
            wtiles[0] = wp.tile([128, KT, 128], mm_dt, name="wtile", tag="w")
            for q in range(4):
                nc.sync.dma_start(
                    out=wtiles[0][:, q * 4 : (q + 1) * 4, :],
                    in_=wt_d[0, :, q * 4 : (q + 1) * 4, :],
                )

            btile = bp.tile([128, OT], f32, name="btile")
            nc.sync.dma_start(out=btile, in_=b_d[:, :])

            xres = []
            xmm = []
            for kt in range(KT):
                t = xp.tile([128, np_tokens], xt_dt, name=f"xr{kt}", tag=f"xr{kt}")
                nc.sync.dma_start(out=t, in_=xt_d[kt * 128 : (kt + 1) * 128, :])
                xres.append(t.bitcast(f32) if mode == "f32r" else t)
                if sixteen_bit:
                    tm = xp.tile(
                        [128, np_tokens], mm_dt, name=f"xm{kt}", tag=f"xm{kt}"
                    )
                    nc.sync.dma_start(out=tm, in_=xtm_d[kt * 128 : (kt + 1) * 128, :])
                    xmm.append(tm)
                else:
                    xmm.append(t)
                if kt == 1:
                    # prefetch wt[1] early, between x tiles
                    wtiles[1] = wp.tile(
                        [128, KT, 128], mm_dt, name="wtile", tag="w"
                    )
                    nc.sync.dma_start(out=wtiles[1], in_=wt_d[1])

            for ot in range(OT):
                if ot in wtiles:
                    wtile = wtiles[ot]
                else:
                    wtile = wp.tile([128, KT, 128], mm_dt, name="wtile", tag="w")
                    nc.sync.dma_start(out=wtile, in_=wt_d[ot])
                otile = op.tile([128, np_tokens], f32, name="otile", tag="o")
                psums = [
                    pp.tile([128, ch], f32, name=f"ps{ci}", tag=f"ps{ci}")
                    for ci, (_, ch) in enumerate(chunks)
                ]
                for kt in range(KT):
                    lhsT = wtile[:, kt, :]
                    for ci, (n0, ch) in enumerate(chunks):
                        nc.tensor.matmul(
                            psums[ci],
                            lhsT,
                            xmm[kt][:, n0 : n0 + ch],
                            start=(kt == 0),
                            stop=(kt == KT - 1),
                        )
                for ci, (n0, ch) in enumerate(chunks):
                    nc.scalar.activation(
                        otile[:, n0 : n0 + ch],
                        psums[ci],
                        relu,
                        bias=btile[:, ot : ot + 1],
                    )
                    nc.vector.tensor_add(
                        otile[:, n0 : n0 + ch],
                        otile[:, n0 : n0 + ch],
                        xres[ot][:, n0 : n0 + ch],
                    )
                    # store per-chunk so the tail DMA overlaps the last
                    # chunks' ACT/DVE instead of waiting for the full tile
                    nc.sync.dma_start(
                        out=out_d[ot * 128 : (ot + 1) * 128, n0 : n0 + ch],
                        in_=otile[:, n0 : n0 + ch],
                    )
    nc.compile()
    return nc


def kernel(x, cam_pred_ids, W, b, _want_results=False):
    x = np.ascontiguousarray(np.asarray(x), dtype=np.float32)
    W = np.asarray(W, dtype=np.float32)
    b = np.asarray(b, dtype=np.float32)
    ids = np.asarray(cam_pred_ids).astype(np.int64)
    batch = x.shape[0]

    counts = np.bincount(ids, minlength=NUM_EXPERTS)
    order = np.argsort(ids, kind="stable")
    np_tokens = max(512, int(counts.max()))

    # per-expert padded token index lists (pad with token 0; discarded later)
    starts = np.zeros(NUM_EXPERTS + 1, dtype=np.int64)
    np.cumsum(counts, out=starts[1:])
    idx = np.zeros((NUM_EXPERTS, np_tokens), dtype=np.int64)
    for e in range(NUM_EXPERTS):
        idx[e, : counts[e]] = order[starts[e] : starts[e + 1]]

    mode = MODE
    mm_np = {
        "f32r": np.float32,
        "f32": np.float32,
        "f16": np.float16,
        "bf16": None,  # ml_dtypes.bfloat16, resolved lazily
    }[mode]
    if mode == "bf16":
        import ml_dtypes

        mm_np = ml_dtypes.bfloat16
    sixteen_bit = mode in ("f16", "bf16")

    in_maps = []
    for e in range(NUM_EXPERTS):
        xg = x[idx[e]]  # [Np, DIM]
        xt = np.ascontiguousarray(xg.T)  # [DIM, Np]
        # wdev[ot, k, kt, o] = W[e][ot*128+o, kt*128+k]
        wdev = np.ascontiguousarray(
            W[e].reshape(OT, 128, KT, 128).transpose(0, 3, 2, 1), dtype=mm_np
        )
        m = {
            "wt": wdev,
            "xt": xt,
            "b": np.ascontiguousarray(b[e].reshape(OT, 128).T),
        }
        if sixteen_bit:
            m["xtm"] = np.ascontiguousarray(xt, dtype=mm_np)
        in_maps.append(m)

    nc = _build_nc(np_tokens, mode)
    res = run_bass_kernel_spmd(
        nc,
        in_maps,
        core_ids=list(range(NUM_EXPERTS)),
        trace=bool(int(os.environ.get("BASS_MOE_TRACE", "0"))),
    )

    out = np.empty_like(x)
    for e in range(NUM_EXPERTS):
        oute = res.results[e]["out"]  # [DIM, Np]
        valid = idx[e, : counts[e]]
        out[valid] = oute.T[: counts[e]]
    if _want_results:
        return out, res
    return out
